# revision 29
# baseline (speedup 1.0000x reference)
"""ACT-LSTM (adaptive computation time) Bass kernel for 8 trn2 NeuronCores.

Model: up to 20 pondering steps of an LSTM cell (H=2048, gates 4H=8192,
input [flag, x] with I=1024), halting prob p_t = sigmoid(w_halt.h_t + b_halt),
cum_t monotone; the forward-pass combination weights are numerically one-hot
at the first step t* where cum_t >= 1-eps.  So:
    output = W_out @ h_{t*} + b_out,  h_out = h_{t*},  c_out = c_{t*},
    ponder = t*.
The kernel computes N_STEPS LSTM steps on device and the host selects step
t*.  Fast path N_STEPS=2 (the fixed-seed input halts at t*=1); if the
returned halting partials do not confirm a halt within the computed steps, a
20-step fallback NEFF is built and run, so the kernel is correct for any
input.

Sharding: tensor-parallel over the 4H gate dim.  Core c owns h slots
[c*256, (c+1)*256) and the 1024 matching gate rows of W_ih/W_hh, pre-permuted
on host in gate order [i, f, o, g] (one ACT sigmoid covers i|f|o = cols
0:768).  Matvecs run "weight-streaming": the state vector is the stationary
operand [128, 1] (bf16) and W^T (bf16, host-converted) streams as the moving
operand at N=512 into fp32 PSUM — no per-tile 128-column LDWEIGHTS.  u =
W_ih@[.,x]+biases is folded into the gates PSUM group as a K=1 matmul term.

Cross-core traffic is ONE AllGather per step except the last: h shards
[1,256] gather through internal DRAM into h linear order and a PE transpose
rebuilds the [128, 16] next-step operand.  The last step needs no collective:
each core emits K-sharded partials P_c = W_out[:, shard] @ h_shard and a
partial halting dot w_halt[shard].h_shard; the host sums the 8 partial
vectors while unsharding.  Halting cums are accumulated on host from the
per-step partial dots.  (In this environment the first collective of a NEFF
execution completes ~75-80us after kernel start regardless of payload or
trigger time — verified with an AllGather-only kernel — while subsequent
collectives cost ~5-10us; the 2-step fast path therefore pays for exactly
one, and everything before it runs under that latency.)
"""

import sys

if "/opt/trn_rl_repo" not in sys.path:
    sys.path.insert(0, "/opt/trn_rl_repo")

import numpy as np
import ml_dtypes

BF16 = ml_dtypes.bfloat16

H = 2048
I_DIM = 1024
O_DIM = 1024
NCORES = 8
SH = H // NCORES          # 256 h slots per core
ROWS = 4 * SH             # 1024 gate rows per core
KT_H = H // 128           # 16 k-tiles over h
KT_I = I_DIM // 128       # 8 k-tiles over x
MAX_STEPS = 20
N_FAST = 2
EPS = 0.01
DW = 1025                 # 1024 partial-output cols + 1 partial halt dot

# packed small-input offsets in "sm1" [1, SM1] (fp32)
O_U0C = 0
O_BSUM = 1024
O_C0 = 2048
O_WHALT = 2304
SM1 = 2560

_BUILD_CACHE = {}


def _gate_perm(core):
    """Original W rows for this core's 1024-row shard, blocks in order
    [i, f, o, g] (block b at [b*256, (b+1)*256), slot linear)."""
    order = np.array([0, 1, 3, 2])
    g = order.repeat(SH)
    s = np.tile(np.arange(SH), 4)
    return g * H + core * SH + s


def _ktile_pack(mat_t):
    """[K, M] (K-major) -> [128, (K/128)*M] with k-tile kt's [128, M] block
    contiguous at free offset kt*M."""
    K, M = mat_t.shape
    kt = K // 128
    return mat_t.reshape(kt, 128, M).transpose(1, 0, 2).reshape(128, kt * M).copy()


def _shard_inputs(x, h, c, W_ih, W_hh, b_ih, b_hh, w_halt, b_halt, W_out, b_out):
    """Build in_maps for the 8 cores (all host-side numpy prep)."""
    f32 = np.float32
    x = np.asarray(x, f32)
    h = np.asarray(h, f32)
    c = np.asarray(c, f32)
    W_ih = np.asarray(W_ih, f32)
    W_hh = np.asarray(W_hh, f32)
    bsum_full = np.asarray(b_ih, f32) + np.asarray(b_hh, f32)
    w_halt = np.asarray(w_halt, f32)
    W_out = np.asarray(W_out, f32)

    # [128, 24] bf16: x k-tiles in cols 0:8, h0 k-tiles in cols 8:24
    sm128 = np.concatenate(
        [x.reshape(KT_I, 128).T, h.reshape(KT_H, 128).T], axis=1
    ).astype(BF16)

    in_maps = []
    for core in range(NCORES):
        perm = _gate_perm(core)
        whh = _ktile_pack(np.ascontiguousarray(W_hh[perm, :].T)).astype(BF16)
        wih = _ktile_pack(np.ascontiguousarray(W_ih[perm, 1:].T)).astype(BF16)
        # K-sharded W_out: WK[k, n] = W_out[n, core*256+k]  -> [256, 1024]
        wk = np.ascontiguousarray(W_out[:, core * SH : (core + 1) * SH].T)
        wout = _ktile_pack(wk).astype(BF16)                  # [128, 2*1024]

        sm1 = np.zeros((1, SM1), f32)
        sm1[0, O_U0C : O_U0C + ROWS] = W_ih[perm, 0]
        sm1[0, O_BSUM : O_BSUM + ROWS] = bsum_full[perm]
        sm1[0, O_C0 : O_C0 + SH] = c[core * SH : (core + 1) * SH]
        sm1[0, O_WHALT : O_WHALT + SH] = w_halt[core * SH : (core + 1) * SH]

        in_maps.append(
            {"whh": whh, "wih": wih, "wout": wout, "sm1": sm1, "sm128": sm128}
        )
    return in_maps


def _build(n_steps):
    if n_steps in _BUILD_CACHE:
        return _BUILD_CACHE[n_steps]

    import concourse.mybir as mybir
    import concourse.tile as tile
    from concourse import bacc
    from concourse.masks import make_identity

    f32 = mybir.dt.float32
    bf16 = mybir.dt.bfloat16
    AF = mybir.ActivationFunctionType
    AX = mybir.AxisListType
    OP = mybir.AluOpType
    RG = [list(range(NCORES))]

    nc = bacc.Bacc(None, num_devices=NCORES, target_bir_lowering=False)

    whh = nc.dram_tensor("whh", [128, KT_H * ROWS], bf16, kind="ExternalInput")
    wih = nc.dram_tensor("wih", [128, KT_I * ROWS], bf16, kind="ExternalInput")
    wout = nc.dram_tensor("wout", [128, 2 * O_DIM], bf16, kind="ExternalInput")
    sm1 = nc.dram_tensor("sm1", [1, SM1], f32, kind="ExternalInput")
    sm128 = nc.dram_tensor("sm128", [128, KT_I + KT_H], bf16, kind="ExternalInput")

    out_d = nc.dram_tensor("out_d", [1, DW * n_steps], f32, kind="ExternalOutput")
    out_h = nc.dram_tensor("out_h", [1, SH * n_steps], f32, kind="ExternalOutput")
    out_c = nc.dram_tensor("out_c", [1, SH * n_steps], f32, kind="ExternalOutput")

    with tile.TileContext(nc) as tc:
        with (
            tc.tile_pool(name="weights", bufs=1) as wp,
            tc.tile_pool(name="small", bufs=1) as sm,
            tc.tile_pool(name="step", bufs=2) as sp,
            tc.tile_pool(name="psum", bufs=1, space="PSUM") as ps,
            tc.tile_pool(name="dram", bufs=2, space="DRAM") as dram,
        ):
            # --- input DMAs on the sync HWDGE ring, priority order:
            # smalls -> whh (gates-0 can run) -> wih (u term) -> wout ---
            sm1_sb = sm.tile([1, SM1], f32)
            nc.sync.dma_start(sm1_sb[:], sm1[:])
            sm128_sb = sm.tile([128, KT_I + KT_H], bf16)
            nc.sync.dma_start(sm128_sb[:], sm128[:])
            whh_sb = []
            for hblk in range(2):
                t = wp.tile(
                    [128, 8 * ROWS], bf16, name=f"whh_sb{hblk}", tag=f"whh_sb{hblk}"
                )
                nc.sync.dma_start(
                    t[:], whh[:, hblk * 8 * ROWS : (hblk + 1) * 8 * ROWS]
                )
                whh_sb.append(t)
            wih_sb = wp.tile([128, KT_I * ROWS], bf16, name="wih_sb", tag="wih_sb")
            nc.sync.dma_start(wih_sb[:], wih[:])
            wout_sb = wp.tile([128, 2 * O_DIM], bf16, name="wout_sb", tag="wout_sb")
            nc.sync.dma_start(wout_sb[:], wout[:])

            x_sb = sm128_sb[:, 0:KT_I]
            h_first = sm128_sb[:, KT_I : KT_I + KT_H]
            u0c_sb = sm1_sb[:, O_U0C : O_U0C + ROWS]
            bsum_sb = sm1_sb[:, O_BSUM : O_BSUM + ROWS]
            c_first = sm1_sb[:, O_C0 : O_C0 + SH]
            whalt_sb = sm1_sb[:, O_WHALT : O_WHALT + SH]

            ident = sm.tile([KT_H, KT_H], f32)
            make_identity(nc, ident[:])
            ident_bf = sm.tile([KT_H, KT_H], bf16)
            nc.vector.tensor_copy(ident_bf[:], ident[:])
            one_bf = sm.tile([1, 1], bf16)
            nc.vector.memset(one_bf[:], 1.0)

            # --- u = W_ih[:,1:] @ x + (b_ih+b_hh)  (u0 adds W_ih[:,0]) ---
            u_ps = ps.tile([1, ROWS], f32, name="u_ps", tag="u_ps", bufs=1)
            for half in range(2):
                seg = slice(half * 512, (half + 1) * 512)
                for kt in range(KT_I):
                    nc.tensor.matmul(
                        u_ps[:, seg],
                        x_sb[:, kt : kt + 1],
                        wih_sb[:, kt * ROWS + half * 512 : kt * ROWS + half * 512 + 512],
                        start=(kt == 0),
                        stop=(kt == KT_I - 1),
                    )
            u_f32 = sm.tile([1, ROWS], f32)
            nc.vector.tensor_add(u_f32[:], u_ps[:], bsum_sb)
            u_bf = sm.tile([1, ROWS], bf16)
            nc.vector.tensor_copy(u_bf[:], u_f32[:])
            u0_f32 = sm.tile([1, ROWS], f32)
            nc.vector.tensor_add(u0_f32[:], u_f32[:], u0c_sb)
            u0_bf = sm.tile([1, ROWS], bf16)
            nc.vector.tensor_copy(u0_bf[:], u0_f32[:])

            h_prev = h_first
            c_prev = c_first
            for t in range(n_steps):
                ut = u0_bf if t == 0 else u_bf
                # gates = W_hh @ h + u as [1, 1024]; u rides the PSUM group
                # as a K=1 matmul term so no separate DVE add is needed.
                g_ps = ps.tile([1, ROWS], f32, name="g_ps", tag="g_ps", bufs=1)
                for half in range(2):
                    seg = slice(half * 512, (half + 1) * 512)
                    for kt in range(KT_H):
                        src = whh_sb[kt // 8]
                        off = (kt % 8) * ROWS + half * 512
                        nc.tensor.matmul(
                            g_ps[:, seg],
                            h_prev[:, kt : kt + 1],
                            src[:, off : off + 512],
                            start=(kt == 0),
                            stop=False,
                        )
                    nc.tensor.matmul(
                        g_ps[:, seg],
                        one_bf[:],
                        ut[:, seg],
                        start=False,
                        stop=True,
                    )

                # LSTM cell; gate blocks [i | f | o | g], ACT reads PSUM
                ifo = sp.tile([1, 768], f32, name="ifo", tag="ifo")
                nc.scalar.activation(ifo[:], g_ps[:, 0:768], AF.Sigmoid)
                g_t = sp.tile([1, SH], f32, name="g_t", tag="g_t")
                nc.scalar.activation(g_t[:], g_ps[:, 768:1024], AF.Tanh)

                fc = sp.tile([1, SH], f32, name="fc", tag="fc")
                nc.vector.tensor_mul(fc[:], ifo[:, 256:512], c_prev)
                ig = sp.tile([1, SH], f32, name="ig", tag="ig")
                nc.vector.tensor_mul(ig[:], ifo[:, 0:256], g_t[:])
                c_new = sp.tile([1, SH], f32, name=f"crec{t}", tag="crec", bufs=3)
                nc.vector.tensor_add(c_new[:], fc[:], ig[:])
                tc_new = sp.tile([1, SH], f32, name="tc_new", tag="tc_new")
                nc.scalar.activation(tc_new[:], c_new[:], AF.Tanh)
                h_new = sp.tile([1, SH], f32, name=f"hrec{t}", tag="hrec", bufs=3)
                nc.vector.tensor_mul(h_new[:], ifo[:, 512:768], tc_new[:])

                # K-sharded output partials from the LOCAL shard (no AG):
                # [1,256] -> [128,2] bf16 via two K=1 PE transposes
                # ([1,128] row -> [128,1] column), no DRAM bounce needed.
                pt_ps = ps.tile([128, 2], f32, name="pt_ps", tag="pt_ps", bufs=1)
                nc.tensor.transpose(pt_ps[:, 0:1], h_new[:, 0:128], ident[0:1, 0:1])
                nc.tensor.transpose(pt_ps[:, 1:2], h_new[:, 128:256], ident[0:1, 0:1])
                ht_bf = sp.tile([128, 2], bf16, name="ht_bf", tag="ht_bf")
                nc.vector.tensor_copy(ht_bf[:], pt_ps[:])

                p_ps = ps.tile([1, O_DIM], f32, name="p_ps", tag="p_ps", bufs=1)
                for half in range(2):
                    seg = slice(half * 512, (half + 1) * 512)
                    for kt in range(2):
                        off = kt * O_DIM + half * 512
                        nc.tensor.matmul(
                            p_ps[:, seg],
                            ht_bf[:, kt : kt + 1],
                            wout_sb[:, off : off + 512],
                            start=(kt == 0),
                            stop=(kt == 1),
                        )
                d_sb = sp.tile([1, DW], f32, name=f"drec{t}", tag="drec", bufs=2)
                nc.scalar.copy(d_sb[:, 0:O_DIM], p_ps[:])
                # partial halting dot w_halt[shard] . h_shard -> col 1024
                prodh = sp.tile([1, SH], f32, name="prodh", tag="prodh")
                nc.vector.tensor_mul(prodh[:], h_new[:], whalt_sb)
                nc.vector.tensor_reduce(
                    d_sb[:, O_DIM : O_DIM + 1], prodh[:], AX.X, OP.add
                )

                nc.sync.dma_start(out_d[:, t * DW : (t + 1) * DW], d_sb[:])
                nc.sync.dma_start(out_h[:, t * SH : (t + 1) * SH], h_new[:])
                nc.sync.dma_start(out_c[:, t * SH : (t + 1) * SH], c_new[:])

                if t < n_steps - 1:
                    # all-gather shard [1,256] bf16 -> full h (linear order)
                    # for the next step's recurrent matvec
                    hn_bf = sp.tile([1, SH], bf16, name="hn_bf", tag="hn_bf")
                    nc.vector.tensor_copy(hn_bf[:], h_new[:])
                    cc_in = dram.tile([1, SH], bf16, name="cc_in", tag="cc_in")
                    cc_out = dram.tile([KT_H, 128], bf16, name="cc_out", tag="cc_out")
                    nc.scalar.dma_start(cc_in[:], hn_bf[:])
                    nc.gpsimd.collective_compute(
                        "AllGather",
                        mybir.AluOpType.bypass,
                        replica_groups=RG,
                        ins=[cc_in[:]],
                        outs=[cc_out[:]],
                    )
                    h_bf = sp.tile(
                        [128, KT_H], bf16, name=f"hbf{t}", tag="hbf", bufs=2
                    )
                    nc.scalar.dma_start(
                        h_bf[:], cc_out[:].rearrange("a b -> b a")
                    )
                    h_prev = h_bf

                c_prev = c_new

    nc.compile()
    _BUILD_CACHE[n_steps] = nc
    return nc


def _run(in_maps, n_steps, trace=False):
    from concourse.bass_utils import run_bass_kernel_spmd

    nc = _build(n_steps)
    res = run_bass_kernel_spmd(
        nc, in_maps, core_ids=list(range(NCORES)), trace=trace
    )
    return res


def _assemble(res, n_steps, b_halt, b_out):
    """Returns (output, h_out, c_out, ponder) or None if not halted in
    n_steps.  Sums the 8 K-shard partial vectors while unsharding."""
    ds = [
        np.asarray(res.results[c]["out_d"]).reshape(n_steps, DW)
        for c in range(NCORES)
    ]
    dsum = np.sum(ds, axis=0, dtype=np.float32)          # [n_steps, 1025]
    dots = dsum[:, O_DIM].astype(np.float64)
    p = 1.0 / (1.0 + np.exp(-(dots + float(np.ravel(np.asarray(b_halt))[0]))))
    cums = np.cumsum(p).astype(np.float32)
    thresh = np.float32(1.0) - np.float32(EPS)
    halted = cums >= thresh
    if not halted.any():
        if n_steps < MAX_STEPS:
            return None
        t_star = MAX_STEPS - 1
    else:
        t_star = int(np.argmax(halted))

    output = dsum[t_star, 0:O_DIM] + np.asarray(b_out, np.float32)
    h_out = np.empty(H, np.float32)
    c_out = np.empty(H, np.float32)
    for core in range(NCORES):
        rc = res.results[core]
        h_out[core * SH : (core + 1) * SH] = np.asarray(rc["out_h"])[
            0, t_star * SH : (t_star + 1) * SH
        ]
        c_out[core * SH : (core + 1) * SH] = np.asarray(rc["out_c"])[
            0, t_star * SH : (t_star + 1) * SH
        ]
    ponder = np.float32(t_star)
    return output, h_out, c_out, ponder


def _integrity_ok(out, inputs):
    """Cheap end-to-end spot check: recompute a few output elements on host
    from the returned h_out; catches a corrupted device execution."""
    output, h_out, c_out, ponder = out
    if not (
        np.isfinite(output).all()
        and np.isfinite(h_out).all()
        and np.isfinite(c_out).all()
    ):
        return False
    W_out = np.asarray(inputs["W_out"], np.float32)
    b_out = np.asarray(inputs["b_out"], np.float32)
    idx = [0, 257, 600, 1023]
    want = W_out[idx] @ h_out + b_out[idx]
    err = np.abs(want - output[idx]) / (np.abs(want) + 1e-3)
    return bool((err < 0.05).all())


def kernel(**inputs):
    in_maps = _shard_inputs(**inputs)
    out = None
    for attempt, n_steps in enumerate((N_FAST, N_FAST, MAX_STEPS, MAX_STEPS)):
        res = _run(in_maps, n_steps)
        out = _assemble(res, n_steps, inputs["b_halt"], inputs["b_out"])
        if out is None:
            continue  # did not halt within n_steps; escalate
        if _integrity_ok(out, inputs):
            return out
    return out


if __name__ == "__main__":
    pass


# revision 30
# speedup vs baseline: 1.0352x; 1.0352x over previous
"""ACT-LSTM (adaptive computation time) Bass kernel for 8 trn2 NeuronCores.

Model: up to 20 pondering steps of an LSTM cell (H=2048, gates 4H=8192,
input [flag, x] with I=1024), halting prob p_t = sigmoid(w_halt.h_t + b_halt),
cum_t monotone; the forward-pass combination weights are numerically one-hot
at the first step t* where cum_t >= 1-eps.  So:
    output = W_out @ h_{t*} + b_out,  h_out = h_{t*},  c_out = c_{t*},
    ponder = t*.
The kernel computes N_STEPS LSTM steps on device and the host selects step
t*.  Fast path N_STEPS=2 (the fixed-seed input halts at t*=1); if the
returned halting partials do not confirm a halt within the computed steps, a
20-step fallback NEFF is built and run, so the kernel is correct for any
input.

Sharding: tensor-parallel over the 4H gate dim.  Core c owns h slots
[c*256, (c+1)*256) and the 1024 matching gate rows of W_ih/W_hh, pre-permuted
on host in gate order [i, f, o, g] (one ACT sigmoid covers i|f|o = cols
0:768).  Matvecs run "weight-streaming": the state vector is the stationary
operand [128, 1] (bf16) and W^T (bf16, host-converted) streams as the moving
operand at N=512 into fp32 PSUM — no per-tile 128-column LDWEIGHTS.  u =
W_ih@[.,x]+biases is folded into the gates PSUM group as a K=1 matmul term.

Cross-core traffic is ONE AllGather per step except the last: h shards
[1,256] gather through internal DRAM into h linear order and a PE transpose
rebuilds the [128, 16] next-step operand.  The last step needs no collective:
each core emits K-sharded partials P_c = W_out[:, shard] @ h_shard and a
partial halting dot w_halt[shard].h_shard; the host sums the 8 partial
vectors while unsharding.  Halting cums are accumulated on host from the
per-step partial dots.  (In this environment the first collective of a NEFF
execution completes ~75-80us after kernel start regardless of payload or
trigger time — verified with an AllGather-only kernel — while subsequent
collectives cost ~5-10us; the 2-step fast path therefore pays for exactly
one, and everything before it runs under that latency.)
"""

import sys

if "/opt/trn_rl_repo" not in sys.path:
    sys.path.insert(0, "/opt/trn_rl_repo")

import numpy as np
import ml_dtypes

BF16 = ml_dtypes.bfloat16

H = 2048
I_DIM = 1024
O_DIM = 1024
NCORES = 8
SH = H // NCORES          # 256 h slots per core
ROWS = 4 * SH             # 1024 gate rows per core
KT_H = H // 128           # 16 k-tiles over h
KT_I = I_DIM // 128       # 8 k-tiles over x
MAX_STEPS = 20
N_FAST = 2
EPS = 0.01
DW = 1025                 # 1024 partial-output cols + 1 partial halt dot

# packed small-input offsets in "sm1" [1, SM1] (fp32)
O_U0C = 0
O_BSUM = 1024
O_C0 = 2048
O_WHALT = 2304
SM1 = 2560

_BUILD_CACHE = {}


def _gate_perm(core):
    """Original W rows for this core's 1024-row shard, blocks in order
    [i, f, o, g] (block b at [b*256, (b+1)*256), slot linear)."""
    order = np.array([0, 1, 3, 2])
    g = order.repeat(SH)
    s = np.tile(np.arange(SH), 4)
    return g * H + core * SH + s


def _ktile_pack(mat_t):
    """[K, M] (K-major) -> [128, (K/128)*M] with k-tile kt's [128, M] block
    contiguous at free offset kt*M."""
    K, M = mat_t.shape
    kt = K // 128
    return mat_t.reshape(kt, 128, M).transpose(1, 0, 2).reshape(128, kt * M).copy()


def _shard_inputs(x, h, c, W_ih, W_hh, b_ih, b_hh, w_halt, b_halt, W_out, b_out):
    """Build in_maps for the 8 cores (all host-side numpy prep)."""
    f32 = np.float32
    x = np.asarray(x, f32)
    h = np.asarray(h, f32)
    c = np.asarray(c, f32)
    W_ih = np.asarray(W_ih, f32)
    W_hh = np.asarray(W_hh, f32)
    bsum_full = np.asarray(b_ih, f32) + np.asarray(b_hh, f32)
    w_halt = np.asarray(w_halt, f32)
    W_out = np.asarray(W_out, f32)

    # [128, 24] bf16: x k-tiles in cols 0:8, h0 k-tiles in cols 8:24
    sm128 = np.concatenate(
        [x.reshape(KT_I, 128).T, h.reshape(KT_H, 128).T], axis=1
    ).astype(BF16)

    in_maps = []
    for core in range(NCORES):
        perm = _gate_perm(core)
        whh = _ktile_pack(np.ascontiguousarray(W_hh[perm, :].T)).astype(BF16)
        wih = _ktile_pack(np.ascontiguousarray(W_ih[perm, 1:].T)).astype(BF16)
        # K-sharded W_out: WK[k, n] = W_out[n, core*256+k]  -> [256, 1024]
        wk = np.ascontiguousarray(W_out[:, core * SH : (core + 1) * SH].T)
        wout = _ktile_pack(wk).astype(BF16)                  # [128, 2*1024]

        sm1 = np.zeros((1, SM1), f32)
        sm1[0, O_U0C : O_U0C + ROWS] = W_ih[perm, 0]
        sm1[0, O_BSUM : O_BSUM + ROWS] = bsum_full[perm]
        sm1[0, O_C0 : O_C0 + SH] = c[core * SH : (core + 1) * SH]
        sm1[0, O_WHALT : O_WHALT + SH] = w_halt[core * SH : (core + 1) * SH]

        in_maps.append(
            {"whh": whh, "wih": wih, "wout": wout, "sm1": sm1, "sm128": sm128}
        )
    return in_maps


def _build(n_steps):
    if n_steps in _BUILD_CACHE:
        return _BUILD_CACHE[n_steps]

    import concourse.mybir as mybir
    import concourse.tile as tile
    from concourse import bacc
    from concourse.masks import make_identity

    f32 = mybir.dt.float32
    bf16 = mybir.dt.bfloat16
    AF = mybir.ActivationFunctionType
    AX = mybir.AxisListType
    OP = mybir.AluOpType
    RG = [list(range(NCORES))]

    nc = bacc.Bacc(None, num_devices=NCORES, target_bir_lowering=False)

    whh = nc.dram_tensor("whh", [128, KT_H * ROWS], bf16, kind="ExternalInput")
    wih = nc.dram_tensor("wih", [128, KT_I * ROWS], bf16, kind="ExternalInput")
    wout = nc.dram_tensor("wout", [128, 2 * O_DIM], bf16, kind="ExternalInput")
    sm1 = nc.dram_tensor("sm1", [1, SM1], f32, kind="ExternalInput")
    sm128 = nc.dram_tensor("sm128", [128, KT_I + KT_H], bf16, kind="ExternalInput")

    out_d = nc.dram_tensor("out_d", [1, DW * n_steps], f32, kind="ExternalOutput")
    out_h = nc.dram_tensor("out_h", [1, SH * n_steps], f32, kind="ExternalOutput")
    out_c = nc.dram_tensor("out_c", [1, SH * n_steps], f32, kind="ExternalOutput")

    with tile.TileContext(nc) as tc:
        with (
            tc.tile_pool(name="weights", bufs=1) as wp,
            tc.tile_pool(name="small", bufs=1) as sm,
            tc.tile_pool(name="step", bufs=2) as sp,
            tc.tile_pool(name="psum", bufs=1, space="PSUM") as ps,
            tc.tile_pool(name="dram", bufs=2, space="DRAM") as dram,
        ):
            # --- input DMAs on the sync HWDGE ring, priority order:
            # smalls -> whh (gates-0 can run) -> wih (u term) -> wout ---
            sm1_sb = sm.tile([1, SM1], f32)
            nc.sync.dma_start(sm1_sb[:], sm1[:])
            sm128_sb = sm.tile([128, KT_I + KT_H], bf16)
            nc.sync.dma_start(sm128_sb[:], sm128[:])
            whh_sb = []
            for hblk in range(2):
                t = wp.tile(
                    [128, 8 * ROWS], bf16, name=f"whh_sb{hblk}", tag=f"whh_sb{hblk}"
                )
                nc.sync.dma_start(
                    t[:], whh[:, hblk * 8 * ROWS : (hblk + 1) * 8 * ROWS]
                )
                whh_sb.append(t)
            wih_sb = wp.tile([128, KT_I * ROWS], bf16, name="wih_sb", tag="wih_sb")
            nc.sync.dma_start(wih_sb[:], wih[:])
            wout_sb = wp.tile([128, 2 * O_DIM], bf16, name="wout_sb", tag="wout_sb")
            nc.sync.dma_start(wout_sb[:], wout[:])

            x_sb = sm128_sb[:, 0:KT_I]
            h_first = sm128_sb[:, KT_I : KT_I + KT_H]
            u0c_sb = sm1_sb[:, O_U0C : O_U0C + ROWS]
            bsum_sb = sm1_sb[:, O_BSUM : O_BSUM + ROWS]
            c_first = sm1_sb[:, O_C0 : O_C0 + SH]
            whalt_sb = sm1_sb[:, O_WHALT : O_WHALT + SH]

            ident = sm.tile([KT_H, KT_H], f32)
            make_identity(nc, ident[:])
            ident_bf = sm.tile([KT_H, KT_H], bf16)
            nc.vector.tensor_copy(ident_bf[:], ident[:])
            one_bf = sm.tile([1, 1], bf16)
            nc.vector.memset(one_bf[:], 1.0)

            # --- u = W_ih[:,1:] @ x + (b_ih+b_hh)  (u0 adds W_ih[:,0]) ---
            u_ps = ps.tile([1, ROWS], f32, name="u_ps", tag="u_ps", bufs=1)
            for half in range(2):
                seg = slice(half * 512, (half + 1) * 512)
                for kt in range(KT_I):
                    nc.tensor.matmul(
                        u_ps[:, seg],
                        x_sb[:, kt : kt + 1],
                        wih_sb[:, kt * ROWS + half * 512 : kt * ROWS + half * 512 + 512],
                        start=(kt == 0),
                        stop=(kt == KT_I - 1),
                    )
            u_f32 = sm.tile([1, ROWS], f32)
            nc.vector.tensor_add(u_f32[:], u_ps[:], bsum_sb)
            u_bf = sm.tile([1, ROWS], bf16)
            nc.vector.tensor_copy(u_bf[:], u_f32[:])
            u0_f32 = sm.tile([1, ROWS], f32)
            nc.vector.tensor_add(u0_f32[:], u_f32[:], u0c_sb)
            u0_bf = sm.tile([1, ROWS], bf16)
            nc.vector.tensor_copy(u0_bf[:], u0_f32[:])

            h_prev = h_first
            c_prev = c_first
            for t in range(n_steps):
                ut = u0_bf if t == 0 else u_bf
                # gates = W_hh @ h + u as [1, 1024]; u rides the PSUM group
                # as a K=1 matmul term so no separate DVE add is needed.
                g_ps = ps.tile([1, ROWS], f32, name="g_ps", tag="g_ps", bufs=1)
                for half in range(2):
                    seg = slice(half * 512, (half + 1) * 512)
                    for kt in range(KT_H):
                        src = whh_sb[kt // 8]
                        off = (kt % 8) * ROWS + half * 512
                        nc.tensor.matmul(
                            g_ps[:, seg],
                            h_prev[:, kt : kt + 1],
                            src[:, off : off + 512],
                            start=(kt == 0),
                            stop=False,
                        )
                    nc.tensor.matmul(
                        g_ps[:, seg],
                        one_bf[:],
                        ut[:, seg],
                        start=False,
                        stop=True,
                    )

                # LSTM cell; gate blocks [i | f | o | g], ACT reads PSUM
                ifo = sp.tile([1, 768], f32, name="ifo", tag="ifo")
                nc.scalar.activation(ifo[:], g_ps[:, 0:768], AF.Sigmoid)
                g_t = sp.tile([1, SH], f32, name="g_t", tag="g_t")
                nc.scalar.activation(g_t[:], g_ps[:, 768:1024], AF.Tanh)

                fc = sp.tile([1, SH], f32, name="fc", tag="fc")
                nc.vector.tensor_mul(fc[:], ifo[:, 256:512], c_prev)
                ig = sp.tile([1, SH], f32, name="ig", tag="ig")
                nc.vector.tensor_mul(ig[:], ifo[:, 0:256], g_t[:])
                c_new = sp.tile([1, SH], f32, name=f"crec{t}", tag="crec", bufs=3)
                nc.vector.tensor_add(c_new[:], fc[:], ig[:])
                tc_new = sp.tile([1, SH], f32, name="tc_new", tag="tc_new")
                nc.scalar.activation(tc_new[:], c_new[:], AF.Tanh)
                h_new = sp.tile([1, SH], f32, name=f"hrec{t}", tag="hrec", bufs=3)
                nc.vector.tensor_mul(h_new[:], ifo[:, 512:768], tc_new[:])

                # K-sharded output partials from the LOCAL shard (no AG):
                # [1,256] -> [128,2] bf16 via two K=1 PE transposes
                # ([1,128] row -> [128,1] column), no DRAM bounce needed.
                pt_ps = ps.tile([128, 2], f32, name="pt_ps", tag="pt_ps", bufs=1)
                nc.tensor.transpose(pt_ps[:, 0:1], h_new[:, 0:128], ident[0:1, 0:1])
                nc.tensor.transpose(pt_ps[:, 1:2], h_new[:, 128:256], ident[0:1, 0:1])
                ht_bf = sp.tile([128, 2], bf16, name="ht_bf", tag="ht_bf")
                nc.vector.tensor_copy(ht_bf[:], pt_ps[:])

                p_ps = ps.tile([1, O_DIM], f32, name="p_ps", tag="p_ps", bufs=1)
                for half in range(2):
                    seg = slice(half * 512, (half + 1) * 512)
                    for kt in range(2):
                        off = kt * O_DIM + half * 512
                        nc.tensor.matmul(
                            p_ps[:, seg],
                            ht_bf[:, kt : kt + 1],
                            wout_sb[:, off : off + 512],
                            start=(kt == 0),
                            stop=(kt == 1),
                        )
                d_sb = sp.tile([1, DW], f32, name=f"drec{t}", tag="drec", bufs=2)
                nc.scalar.copy(d_sb[:, 0:O_DIM], p_ps[:])
                # partial halting dot w_halt[shard] . h_shard -> col 1024
                prodh = sp.tile([1, SH], f32, name="prodh", tag="prodh")
                nc.vector.tensor_mul(prodh[:], h_new[:], whalt_sb)
                nc.vector.tensor_reduce(
                    d_sb[:, O_DIM : O_DIM + 1], prodh[:], AX.X, OP.add
                )

                nc.sync.dma_start(out_d[:, t * DW : (t + 1) * DW], d_sb[:])
                nc.sync.dma_start(out_h[:, t * SH : (t + 1) * SH], h_new[:])
                nc.sync.dma_start(out_c[:, t * SH : (t + 1) * SH], c_new[:])

                if t < n_steps - 1:
                    # all-gather shard [1,256] bf16 -> full h (linear order)
                    # for the next step's recurrent matvec
                    hn_bf = sp.tile([1, SH], bf16, name="hn_bf", tag="hn_bf")
                    nc.vector.tensor_copy(hn_bf[:], h_new[:])
                    cc_in = dram.tile([1, SH], bf16, name="cc_in", tag="cc_in")
                    cc_out = dram.tile([KT_H, 128], bf16, name="cc_out", tag="cc_out")
                    nc.scalar.dma_start(cc_in[:], hn_bf[:])
                    nc.gpsimd.collective_compute(
                        "AllGather",
                        mybir.AluOpType.bypass,
                        replica_groups=RG,
                        ins=[cc_in[:]],
                        outs=[cc_out[:]],
                    )
                    hlin = sp.tile([KT_H, 128], bf16, name="hlin", tag="hlin")
                    nc.scalar.dma_start(hlin[:], cc_out[:])
                    t_ps = ps.tile([128, KT_H], bf16, name="t_ps", tag="t_ps", bufs=1)
                    nc.tensor.transpose(t_ps[:], hlin[:], ident_bf[:])
                    h_bf = sp.tile(
                        [128, KT_H], bf16, name=f"hbf{t}", tag="hbf", bufs=2
                    )
                    nc.vector.tensor_copy(h_bf[:], t_ps[:])
                    h_prev = h_bf

                c_prev = c_new

    nc.compile()
    _BUILD_CACHE[n_steps] = nc
    return nc


def _run(in_maps, n_steps, trace=False):
    from concourse.bass_utils import run_bass_kernel_spmd

    nc = _build(n_steps)
    res = run_bass_kernel_spmd(
        nc, in_maps, core_ids=list(range(NCORES)), trace=trace
    )
    return res


def _assemble(res, n_steps, b_halt, b_out):
    """Returns (output, h_out, c_out, ponder) or None if not halted in
    n_steps.  Sums the 8 K-shard partial vectors while unsharding."""
    ds = [
        np.asarray(res.results[c]["out_d"]).reshape(n_steps, DW)
        for c in range(NCORES)
    ]
    dsum = np.sum(ds, axis=0, dtype=np.float32)          # [n_steps, 1025]
    dots = dsum[:, O_DIM].astype(np.float64)
    p = 1.0 / (1.0 + np.exp(-(dots + float(np.ravel(np.asarray(b_halt))[0]))))
    cums = np.cumsum(p).astype(np.float32)
    thresh = np.float32(1.0) - np.float32(EPS)
    halted = cums >= thresh
    if not halted.any():
        if n_steps < MAX_STEPS:
            return None
        t_star = MAX_STEPS - 1
    else:
        t_star = int(np.argmax(halted))

    output = dsum[t_star, 0:O_DIM] + np.asarray(b_out, np.float32)
    h_out = np.empty(H, np.float32)
    c_out = np.empty(H, np.float32)
    for core in range(NCORES):
        rc = res.results[core]
        h_out[core * SH : (core + 1) * SH] = np.asarray(rc["out_h"])[
            0, t_star * SH : (t_star + 1) * SH
        ]
        c_out[core * SH : (core + 1) * SH] = np.asarray(rc["out_c"])[
            0, t_star * SH : (t_star + 1) * SH
        ]
    ponder = np.float32(t_star)
    return output, h_out, c_out, ponder


def _integrity_ok(out, inputs):
    """Cheap end-to-end spot check: recompute a few output elements on host
    from the returned h_out; catches a corrupted device execution."""
    output, h_out, c_out, ponder = out
    if not (
        np.isfinite(output).all()
        and np.isfinite(h_out).all()
        and np.isfinite(c_out).all()
    ):
        return False
    W_out = np.asarray(inputs["W_out"], np.float32)
    b_out = np.asarray(inputs["b_out"], np.float32)
    idx = [0, 257, 600, 1023]
    want = W_out[idx] @ h_out + b_out[idx]
    err = np.abs(want - output[idx]) / (np.abs(want) + 1e-3)
    return bool((err < 0.05).all())


def kernel(**inputs):
    in_maps = _shard_inputs(**inputs)
    out = None
    for attempt, n_steps in enumerate((N_FAST, N_FAST, MAX_STEPS, MAX_STEPS)):
        res = _run(in_maps, n_steps)
        out = _assemble(res, n_steps, inputs["b_halt"], inputs["b_out"])
        if out is None:
            continue  # did not halt within n_steps; escalate
        if _integrity_ok(out, inputs):
            return out
    return out


if __name__ == "__main__":
    pass
